# revision 3
# baseline (speedup 1.0000x reference)
"""AttentionReadout kernel for 8 Trainium2 NeuronCores.

Math (per graph g): pooled[g] = sum_i attn_i * x_i with
  attn_i = e_i / sum_{j in g} e_j,  e_i = exp(tanh(x_i @ W1 + b1) @ W2 + b2)
  out = pooled @ Wt + bt

Sharding: graph-aligned data parallel. Core k owns graphs [128k, 128k+128)
and exactly the (contiguous, since batch is sorted) nodes of those graphs.
Each core computes its own 128 graphs end-to-end; no collectives. Host
concatenates the 8 [128, 128] output shards.

Device data flow per 512-node macro tile:
  - plain DMA loads x tiles natural [n, c] (bf16) into a [128, 4, 129]
    layout whose last column is constant 1.0 (gives the softmax denominator
    for free in the pooled matmul, N=129).
  - hardware DMA-transpose loads the same DRAM range as [c, n] (bf16).
  - PE: hT[64,512] = W1b.T @ xT;  ACT: h = tanh(hT + b1) (bf16)
  - PE per 128-chunk: scores[n,1] = h_chunk.T @ W2  (lhsT = h chunk)
  - ACT: e[128,4] = exp(scores + b2)
  - DVE per chunk: ohe[n,g] = (iota == gid) * e   (one fused tensor_scalar)
  - PE per chunk: pacc[g, 0:129] += ohe.T @ [x | 1]  (start/stop over all
    chunks; column 128 accumulates den[g])
Final: den=pacc[:,128]; pooled_n = pacc[:,0:128]/den (DVE); PE transpose;
outT[o,g] = Wt.T @ pooled_n.T + bt; DMA out.
"""

import numpy as np
import ml_dtypes
from contextlib import ExitStack

import concourse.bass as bass
import concourse.bacc as bacc
import concourse.tile as tile
from concourse import mybir
from concourse.bass_utils import run_bass_kernel_spmd

N_CORES = 8
G = 1024
GPC = G // N_CORES  # 128 graphs per core
IN_C = 128
HID = 64
OUT_C = 128
MACRO = 512          # nodes per macro tile
CHUNK = 128          # nodes per chunk (PE contraction width)
BF16 = mybir.dt.bfloat16
F32 = mybir.dt.float32

_CACHE = {}


def _build(npad):
    nm = npad // CHUNK        # gid columns
    n_macros = npad // MACRO

    nc = bacc.Bacc("TRN2", target_bir_lowering=False, debug=False,
                   num_devices=N_CORES)

    x_nat = nc.dram_tensor("x_nat", [npad, IN_C], BF16, kind="ExternalInput").ap()
    gid_d = nc.dram_tensor("gid", [CHUNK, nm], F32, kind="ExternalInput").ap()
    iota_d = nc.dram_tensor("iota", [CHUNK, GPC], BF16, kind="ExternalInput").ap()
    w1_d = nc.dram_tensor("w1", [IN_C, HID], BF16, kind="ExternalInput").ap()
    b1_d = nc.dram_tensor("b1", [HID, 1], F32, kind="ExternalInput").ap()
    w2_d = nc.dram_tensor("w2", [HID, 1], BF16, kind="ExternalInput").ap()
    b2_d = nc.dram_tensor("b2", [CHUNK, 1], F32, kind="ExternalInput").ap()
    wt_d = nc.dram_tensor("wt", [IN_C, OUT_C], F32, kind="ExternalInput").ap()
    bt_d = nc.dram_tensor("bt", [OUT_C, 1], F32, kind="ExternalInput").ap()
    id_d = nc.dram_tensor("idm", [128, 128], F32, kind="ExternalInput").ap()
    out_d = nc.dram_tensor("outT", [OUT_C, GPC], F32, kind="ExternalOutput").ap()

    with tile.TileContext(nc) as tc, ExitStack() as ctx:
        consts = ctx.enter_context(tc.tile_pool(name="consts", bufs=1))
        xn_pool = ctx.enter_context(tc.tile_pool(name="xn", bufs=4))
        xt_pool = ctx.enter_context(tc.tile_pool(name="xt", bufs=4))
        hb_pool = ctx.enter_context(tc.tile_pool(name="hb", bufs=3))
        e4_pool = ctx.enter_context(tc.tile_pool(name="e4", bufs=3))
        ohe_pool = ctx.enter_context(tc.tile_pool(name="ohe", bufs=6))
        fin_pool = ctx.enter_context(tc.tile_pool(name="fin", bufs=1))
        ph_pool = ctx.enter_context(tc.tile_pool(name="ph", bufs=2, space="PSUM"))
        ps_pool = ctx.enter_context(tc.tile_pool(name="ps", bufs=2, space="PSUM"))
        pacc_pool = ctx.enter_context(tc.tile_pool(name="pacc", bufs=1, space="PSUM"))
        pfin_pool = ctx.enter_context(tc.tile_pool(name="pfin", bufs=1, space="PSUM"))

        # constants
        gid_s = consts.tile([CHUNK, nm], F32)
        nc.sync.dma_start(gid_s[:], gid_d[:])
        iota_s = consts.tile([CHUNK, GPC], BF16)
        nc.sync.dma_start(iota_s[:], iota_d[:])
        w1_s = consts.tile([IN_C, HID], BF16)
        nc.sync.dma_start(w1_s[:], w1_d[:])
        b1_s = consts.tile([HID, 1], F32)
        nc.sync.dma_start(b1_s[:], b1_d[:])
        w2_s = consts.tile([HID, 1], BF16)
        nc.sync.dma_start(w2_s[:], w2_d[:])
        b2_s = consts.tile([CHUNK, 1], F32)
        nc.sync.dma_start(b2_s[:], b2_d[:])
        wt_s = consts.tile([IN_C, OUT_C], F32)
        nc.sync.dma_start(wt_s[:], wt_d[:])
        bt_s = consts.tile([OUT_C, 1], F32)
        nc.sync.dma_start(bt_s[:], bt_d[:])
        id_s = consts.tile([128, 128], F32)
        nc.sync.dma_start(id_s[:], id_d[:])

        pacc = pacc_pool.tile([GPC, IN_C + 1], F32)  # [g, c | den]

        n_chunks_total = n_macros * (MACRO // CHUNK)
        ci = 0  # global chunk counter
        for m in range(n_macros):
            n0 = m * MACRO
            # natural-layout load: [128, 4, 129] (col 128 = ones)
            xn = xn_pool.tile([CHUNK, MACRO // CHUNK, IN_C + 1], BF16)
            nc.sync.dma_start(
                xn[:, :, 0:IN_C],
                x_nat[n0:n0 + MACRO, :].rearrange("(j p) c -> p j c", p=CHUNK),
            )
            nc.gpsimd.memset(xn[:, :, IN_C:IN_C + 1], 1.0)
            # transposed load via DMA xbar: [c, n]
            xt = xt_pool.tile([IN_C, MACRO], BF16)
            nc.scalar.dma_start(xt[:], x_nat[n0:n0 + MACRO, :], transpose=True)

            # MLP layer 1: hT = W1.T @ xT -> [64, 512]
            ph = ph_pool.tile([HID, MACRO], F32)
            nc.tensor.matmul(ph[:], w1_s[:], xt[:], start=True, stop=True)
            hb = hb_pool.tile([HID, MACRO], BF16)
            nc.scalar.activation(hb[:], ph[:],
                                 mybir.ActivationFunctionType.Tanh, bias=b1_s[:])

            # scores per chunk -> [128, 4] psum
            ps = ps_pool.tile([CHUNK, MACRO // CHUNK], F32)
            for j in range(MACRO // CHUNK):
                nc.tensor.matmul(ps[:, j:j + 1],
                                 hb[:, j * CHUNK:(j + 1) * CHUNK], w2_s[:],
                                 start=True, stop=True)
            e4 = e4_pool.tile([CHUNK, MACRO // CHUNK], F32)
            nc.scalar.activation(e4[:], ps[:],
                                 mybir.ActivationFunctionType.Exp, bias=b2_s[:])

            for j in range(MACRO // CHUNK):
                q = m * (MACRO // CHUNK) + j
                ohe = ohe_pool.tile([CHUNK, GPC], BF16)
                nc.vector.tensor_scalar(
                    ohe[:], iota_s[:],
                    gid_s[:, q:q + 1], e4[:, j:j + 1],
                    mybir.AluOpType.is_equal, mybir.AluOpType.mult)
                nc.tensor.matmul(pacc[:], ohe[:], xn[:, j, :],
                                 start=(ci == 0), stop=(ci == n_chunks_total - 1))
                ci += 1

        # ---- final: normalize, transform, write out ----
        rden = fin_pool.tile([GPC, 1], F32, tag="rden")
        nc.vector.reciprocal(rden[:], pacc[:, IN_C:IN_C + 1])
        pooln = fin_pool.tile([GPC, IN_C], F32, tag="pooln")
        nc.vector.tensor_scalar(pooln[:], pacc[:, 0:IN_C], rden[:], None,
                                mybir.AluOpType.mult)
        ptr = pfin_pool.tile([IN_C, GPC], F32)
        nc.tensor.transpose(ptr[:], pooln[:], id_s[:])
        poolT = fin_pool.tile([IN_C, GPC], F32, tag="poolT")
        nc.scalar.copy(poolT[:], ptr[:])
        pfin = pfin_pool.tile([OUT_C, GPC], F32)
        nc.tensor.matmul(pfin[:], wt_s[:], poolT[:], start=True, stop=True)
        outT_s = fin_pool.tile([OUT_C, GPC], F32, tag="outT")
        nc.scalar.activation(outT_s[:], pfin[:],
                             mybir.ActivationFunctionType.Identity, bias=bt_s[:])
        nc.sync.dma_start(out_d[:], outT_s[:])

    nc.compile()
    return nc


def kernel(x, batch, W1, b1, W2, b2, Wt, bt, _trace=False, _trace_kwargs=None):
    x = np.asarray(x)
    batch = np.asarray(batch)
    W1 = np.asarray(W1, dtype=np.float32)
    b1 = np.asarray(b1, dtype=np.float32)
    W2 = np.asarray(W2, dtype=np.float32)
    b2 = np.asarray(b2, dtype=np.float32)
    Wt = np.asarray(Wt, dtype=np.float32)
    bt = np.asarray(bt, dtype=np.float32)

    n = x.shape[0]
    starts = np.searchsorted(batch, np.arange(N_CORES + 1) * GPC).astype(np.int64)
    counts = np.diff(starts)
    npad = int(-(-counts.max() // MACRO) * MACRO)
    nm = npad // CHUNK

    key = npad
    if key not in _CACHE:
        _CACHE[key] = _build(npad)
    nc = _CACHE[key]

    bf16 = ml_dtypes.bfloat16
    iota = np.broadcast_to(np.arange(GPC, dtype=np.float32), (CHUNK, GPC))
    common = {
        "iota": iota.astype(bf16),
        "w1": W1.astype(bf16),
        "b1": b1.reshape(HID, 1).astype(np.float32),
        "w2": W2.reshape(HID, 1).astype(bf16),
        "b2": np.full((CHUNK, 1), float(b2.ravel()[0]), dtype=np.float32),
        "wt": Wt.astype(np.float32),
        "bt": bt.reshape(OUT_C, 1).astype(np.float32),
        "idm": np.eye(128, dtype=np.float32),
    }
    in_maps = []
    for k in range(N_CORES):
        s, e = int(starts[k]), int(starts[k + 1])
        cnt = e - s
        x_nat = np.zeros((npad, IN_C), dtype=bf16)
        x_nat[:cnt] = x[s:e].astype(bf16)
        gid_lin = np.full(npad, -1.0, dtype=np.float32)
        gid_lin[:cnt] = (batch[s:e] - k * GPC).astype(np.float32)
        gid = np.ascontiguousarray(gid_lin.reshape(nm, CHUNK).T)
        in_maps.append({"x_nat": x_nat, "gid": gid, **common})

    res = run_bass_kernel_spmd(
        nc, in_maps, core_ids=list(range(N_CORES)),
        trace=_trace, **(_trace_kwargs or {}))

    out = np.empty((G, OUT_C), dtype=np.float32)
    for k in range(N_CORES):
        out[k * GPC:(k + 1) * GPC, :] = res.results[k]["outT"].T
    if _trace:
        return out, res
    return out


# revision 5
# speedup vs baseline: 1.9486x; 1.9486x over previous
"""AttentionReadout kernel for 8 Trainium2 NeuronCores.

Math (per graph g): pooled[g] = sum_i attn_i * x_i with
  attn_i = e_i / sum_{j in g} e_j,  e_i = exp(tanh(x_i @ W1 + b1) @ W2 + b2)
  out = pooled @ Wt + bt

Sharding: graph-aligned data parallel. Core k owns graphs [128k, 128k+128)
and exactly the (contiguous, since batch is sorted) nodes of those graphs.
Each core computes its own 128 graphs end-to-end; no collectives. Host
concatenates the 8 [128, 128] output shards.

Device data flow per 512-node macro tile:
  - plain DMA loads x tiles natural [n, c] (bf16) into a [128, 4, 129]
    layout whose last column is constant 1.0 (gives the softmax denominator
    for free in the pooled matmul, N=129).
  - hardware DMA-transpose loads the same DRAM range as [c, n] (bf16).
  - PE: hT[64,512] = W1b.T @ xT;  ACT: h = tanh(hT + b1) (bf16)
  - PE per 128-chunk: scores[n,1] = h_chunk.T @ W2  (lhsT = h chunk)
  - ACT: e[128,4] = exp(scores + b2)
  - DVE per chunk: ohe[n,g] = (iota == gid) * e   (one fused tensor_scalar)
  - PE per chunk: pacc[g, 0:129] += ohe.T @ [x | 1]  (start/stop over all
    chunks; column 128 accumulates den[g])
Final: den=pacc[:,128]; pooled_n = pacc[:,0:128]/den (DVE); PE transpose;
outT[o,g] = Wt.T @ pooled_n.T + bt; DMA out.
"""

import numpy as np
import ml_dtypes
from contextlib import ExitStack

import concourse.bass as bass
import concourse.bacc as bacc
import concourse.tile as tile
from concourse import mybir
from concourse.bass_utils import run_bass_kernel_spmd

N_CORES = 8
G = 1024
GPC = G // N_CORES  # 128 graphs per core
IN_C = 128
HID = 64
OUT_C = 128
MACRO = 512          # nodes per macro tile
DMAT = 2048          # nodes per natural-load DMA tile
TPOSE = 2048         # nodes per DMA-transpose tile
CHUNK = 128          # nodes per chunk (PE contraction width)
BF16 = mybir.dt.bfloat16
F32 = mybir.dt.float32

_CACHE = {}


def _build(npad):
    nm = npad // CHUNK        # gid columns
    n_macros = npad // MACRO
    assert npad % TPOSE == 0 or npad % DMAT == 0

    nc = bacc.Bacc("TRN2", target_bir_lowering=False, debug=False,
                   num_devices=N_CORES)

    x_nat = nc.dram_tensor("x_nat", [npad, IN_C], BF16, kind="ExternalInput").ap()
    gid_d = nc.dram_tensor("gid", [CHUNK, nm], F32, kind="ExternalInput").ap()
    iota_d = nc.dram_tensor("iota", [CHUNK, GPC], BF16, kind="ExternalInput").ap()
    w1_d = nc.dram_tensor("w1", [IN_C, HID], BF16, kind="ExternalInput").ap()
    b1_d = nc.dram_tensor("b1", [HID, 1], F32, kind="ExternalInput").ap()
    w2_d = nc.dram_tensor("w2", [HID, 1], BF16, kind="ExternalInput").ap()
    b2_d = nc.dram_tensor("b2", [CHUNK, 1], F32, kind="ExternalInput").ap()
    wt_d = nc.dram_tensor("wt", [IN_C, OUT_C], F32, kind="ExternalInput").ap()
    bt_d = nc.dram_tensor("bt", [OUT_C, 1], F32, kind="ExternalInput").ap()
    id_d = nc.dram_tensor("idm", [128, 128], F32, kind="ExternalInput").ap()
    out_d = nc.dram_tensor("outT", [OUT_C, GPC], F32, kind="ExternalOutput").ap()

    with tile.TileContext(nc) as tc, ExitStack() as ctx:
        consts = ctx.enter_context(tc.tile_pool(name="consts", bufs=1))
        xn_pool = ctx.enter_context(tc.tile_pool(name="xn", bufs=3))
        xt_pool = ctx.enter_context(tc.tile_pool(name="xt", bufs=2))
        hb_pool = ctx.enter_context(tc.tile_pool(name="hb", bufs=4))
        e4_pool = ctx.enter_context(tc.tile_pool(name="e4", bufs=4))
        ohe_pool = ctx.enter_context(tc.tile_pool(name="ohe", bufs=8))
        fin_pool = ctx.enter_context(tc.tile_pool(name="fin", bufs=1))
        ph_pool = ctx.enter_context(tc.tile_pool(name="ph", bufs=2, space="PSUM"))
        ps_pool = ctx.enter_context(tc.tile_pool(name="ps", bufs=2, space="PSUM"))
        pacc_pool = ctx.enter_context(tc.tile_pool(name="pacc", bufs=1, space="PSUM"))
        pfin_pool = ctx.enter_context(tc.tile_pool(name="pfin", bufs=1, space="PSUM"))

        # constants
        gid_s = consts.tile([CHUNK, nm], F32)
        nc.sync.dma_start(gid_s[:], gid_d[:])
        iota_s = consts.tile([CHUNK, GPC], BF16)
        nc.sync.dma_start(iota_s[:], iota_d[:])
        w1_s = consts.tile([IN_C, HID], BF16)
        nc.sync.dma_start(w1_s[:], w1_d[:])
        b1_s = consts.tile([HID, 1], F32)
        nc.sync.dma_start(b1_s[:], b1_d[:])
        w2_s = consts.tile([HID, 1], BF16)
        nc.sync.dma_start(w2_s[:], w2_d[:])
        b2_s = consts.tile([CHUNK, 1], F32)
        nc.sync.dma_start(b2_s[:], b2_d[:])
        wt_s = consts.tile([IN_C, OUT_C], F32)
        nc.sync.dma_start(wt_s[:], wt_d[:])
        bt_s = consts.tile([OUT_C, 1], F32)
        nc.sync.dma_start(bt_s[:], bt_d[:])
        id_s = consts.tile([128, 128], F32)
        nc.sync.dma_start(id_s[:], id_d[:])

        pacc = pacc_pool.tile([GPC, IN_C + 1], F32)  # [g, c | den]

        n_chunks_total = n_macros * (MACRO // CHUNK)
        ci = 0  # global chunk counter
        xn = None
        xt = None
        for m in range(n_macros):
            n0 = m * MACRO
            if n0 % DMAT == 0:
                # natural-layout load: [128, 16, 129] (col 128 = ones)
                nd = min(DMAT, npad - n0)
                xn = xn_pool.tile([CHUNK, DMAT // CHUNK, IN_C + 1], BF16)
                nc.sync.dma_start(
                    xn[:, 0:nd // CHUNK, 0:IN_C],
                    x_nat[n0:n0 + nd, :].rearrange("(j p) c -> p j c", p=CHUNK),
                )
                nc.gpsimd.memset(xn[:, :, IN_C:IN_C + 1], 1.0)
            if n0 % TPOSE == 0:
                # transposed load via DMA xbar: [c, n]
                nt = min(TPOSE, npad - n0)
                xt = xt_pool.tile([IN_C, TPOSE], BF16)
                nc.scalar.dma_start(xt[:, 0:nt], x_nat[n0:n0 + nt, :],
                                    transpose=True)
            mj = (m * MACRO % DMAT) // MACRO     # macro index within xn tile
            mt = (m * MACRO % TPOSE)             # node offset within xt tile

            # MLP layer 1: hT = W1.T @ xT -> [64, 512]
            ph = ph_pool.tile([HID, MACRO], F32)
            nc.tensor.matmul(ph[:], w1_s[:], xt[:, mt:mt + MACRO],
                             start=True, stop=True)
            hb = hb_pool.tile([HID, MACRO], BF16)
            nc.scalar.activation(hb[:], ph[:],
                                 mybir.ActivationFunctionType.Tanh, bias=b1_s[:])

            # scores per chunk -> [128, 4] psum
            ps = ps_pool.tile([CHUNK, MACRO // CHUNK], F32)
            for j in range(MACRO // CHUNK):
                nc.tensor.matmul(ps[:, j:j + 1],
                                 hb[:, j * CHUNK:(j + 1) * CHUNK], w2_s[:],
                                 start=True, stop=True)
            e4 = e4_pool.tile([CHUNK, MACRO // CHUNK], F32)
            nc.scalar.activation(e4[:], ps[:],
                                 mybir.ActivationFunctionType.Exp, bias=b2_s[:])

            for j in range(MACRO // CHUNK):
                q = m * (MACRO // CHUNK) + j
                ohe = ohe_pool.tile([CHUNK, GPC], BF16)
                nc.vector.tensor_scalar(
                    ohe[:], iota_s[:],
                    gid_s[:, q:q + 1], e4[:, j:j + 1],
                    mybir.AluOpType.is_equal, mybir.AluOpType.mult)
                nc.tensor.matmul(pacc[:], ohe[:],
                                 xn[:, mj * (MACRO // CHUNK) + j, :],
                                 start=(ci == 0), stop=(ci == n_chunks_total - 1))
                ci += 1

        # ---- final: normalize, transform, write out ----
        rden = fin_pool.tile([GPC, 1], F32, tag="rden")
        nc.vector.reciprocal(rden[:], pacc[:, IN_C:IN_C + 1])
        pooln = fin_pool.tile([GPC, IN_C], F32, tag="pooln")
        nc.vector.tensor_scalar(pooln[:], pacc[:, 0:IN_C], rden[:], None,
                                mybir.AluOpType.mult)
        ptr = pfin_pool.tile([IN_C, GPC], F32)
        nc.tensor.transpose(ptr[:], pooln[:], id_s[:])
        poolT = fin_pool.tile([IN_C, GPC], F32, tag="poolT")
        nc.scalar.copy(poolT[:], ptr[:])
        pfin = pfin_pool.tile([OUT_C, GPC], F32)
        nc.tensor.matmul(pfin[:], wt_s[:], poolT[:], start=True, stop=True)
        outT_s = fin_pool.tile([OUT_C, GPC], F32, tag="outT")
        nc.scalar.activation(outT_s[:], pfin[:],
                             mybir.ActivationFunctionType.Identity, bias=bt_s[:])
        nc.sync.dma_start(out_d[:], outT_s[:])

    nc.compile()
    return nc


def kernel(x, batch, W1, b1, W2, b2, Wt, bt, _trace=False, _trace_kwargs=None):
    x = np.asarray(x)
    batch = np.asarray(batch)
    W1 = np.asarray(W1, dtype=np.float32)
    b1 = np.asarray(b1, dtype=np.float32)
    W2 = np.asarray(W2, dtype=np.float32)
    b2 = np.asarray(b2, dtype=np.float32)
    Wt = np.asarray(Wt, dtype=np.float32)
    bt = np.asarray(bt, dtype=np.float32)

    n = x.shape[0]
    starts = np.searchsorted(batch, np.arange(N_CORES + 1) * GPC).astype(np.int64)
    counts = np.diff(starts)
    npad = int(-(-counts.max() // TPOSE) * TPOSE)
    nm = npad // CHUNK

    key = npad
    if key not in _CACHE:
        _CACHE[key] = _build(npad)
    nc = _CACHE[key]

    bf16 = ml_dtypes.bfloat16
    iota = np.broadcast_to(np.arange(GPC, dtype=np.float32), (CHUNK, GPC))
    common = {
        "iota": iota.astype(bf16),
        "w1": W1.astype(bf16),
        "b1": b1.reshape(HID, 1).astype(np.float32),
        "w2": W2.reshape(HID, 1).astype(bf16),
        "b2": np.full((CHUNK, 1), float(b2.ravel()[0]), dtype=np.float32),
        "wt": Wt.astype(np.float32),
        "bt": bt.reshape(OUT_C, 1).astype(np.float32),
        "idm": np.eye(128, dtype=np.float32),
    }
    in_maps = []
    for k in range(N_CORES):
        s, e = int(starts[k]), int(starts[k + 1])
        cnt = e - s
        x_nat = np.zeros((npad, IN_C), dtype=bf16)
        x_nat[:cnt] = x[s:e].astype(bf16)
        gid_lin = np.full(npad, -1.0, dtype=np.float32)
        gid_lin[:cnt] = (batch[s:e] - k * GPC).astype(np.float32)
        gid = np.ascontiguousarray(gid_lin.reshape(nm, CHUNK).T)
        in_maps.append({"x_nat": x_nat, "gid": gid, **common})

    res = run_bass_kernel_spmd(
        nc, in_maps, core_ids=list(range(N_CORES)),
        trace=_trace, **(_trace_kwargs or {}))

    out = np.empty((G, OUT_C), dtype=np.float32)
    for k in range(N_CORES):
        out[k * GPC:(k + 1) * GPC, :] = res.results[k]["outT"].T
    if _trace:
        return out, res
    return out
